# revision 22
# baseline (speedup 1.0000x reference)
"""Trainium2 Bass kernel: 4-layer decoder prefill (S=1024, H=2048, NH=16, HD=128,
FFN=5632, V=32000), tensor-parallel over 8 NeuronCores.

v2 design:
- Megatron TP over 8 cores (2 heads / 704 ffn rows / 4000 vocab rows per core).
- Weights converted to fp16 on host (FWL-eligible, half the HBM traffic) and
  pre-chunked so every weight DMA is one large contiguous [128, N] transfer,
  loaded once per use-pass.
- RMSNorm weights folded into wq/wk/wv/w1/w3/out_w on host; the per-token
  1/rms scale is broadcast via a ones-matmul and fused into the PSUM->SBUF
  consume ops, so normalization adds no dedicated passes.
- Causal block-skip: score blocks entirely above the diagonal are skipped,
  blocks fully below skip the mask add; only the 4 diagonal 128x512 mask
  blocks (shared by both token halves) are kept in SBUF.
- Software pipeline over (layer, token-half) chains: A = QKV->RoPE->attn->
  wo->AllReduce1, F = resid-add->norm->SwiGLU->w2->AllReduce2, emitted so
  ~35us of independent PE work covers every collective.
- All on-chip activations and weights are fp16 (PSUM accumulation in
  fp32); per-token 1/rms and softmax 1/sum broadcasts stay fp32.
"""

import os
import sys

sys.path.insert(0, "/opt/trn_rl_repo")

import numpy as np

L, B, S, H, NH, HD = 4, 1, 1024, 2048, 16, 128
V, P = 32000, 5632
NC = 8
FEAT = H // NC          # 256 qkv features per core (2 heads of 128)
PC = P // NC            # 704 ffn rows per core
PCP = 768               # padded to 6 chunks of 128
VC = V // NC            # 4000 vocab rows per core
KH = H // 128           # 16 H-chunks
EPS = 1e-5
INV_SCALE = float(1.0 / np.sqrt(HD))

_STATE = {}


def _build():
    import concourse.bass as bass
    import concourse.bacc as bacc
    from concourse import tile, mybir

    F32 = mybir.dt.float32
    F32R = mybir.dt.float32r
    F16 = mybir.dt.float16
    BF16 = mybir.dt.float16  # all 16-bit on-chip data is fp16
    AF = mybir.ActivationFunctionType
    ALU = mybir.AluOpType

    nc = bacc.Bacc("TRN2", target_bir_lowering=False, debug=False, num_devices=NC)

    xT_h = nc.dram_tensor("xT", [128, KH * S], F16, kind="ExternalInput")
    mask_h = nc.dram_tensor("mask4", [128, 4 * 512], BF16, kind="ExternalInput")
    C_h = nc.dram_tensor("Cb", [128, S], BF16, kind="ExternalInput")
    S_h = nc.dram_tensor("Sb", [128, S], BF16, kind="ExternalInput")
    J_h = nc.dram_tensor("Jb", [128, 128], BF16, kind="ExternalInput")
    id_h = nc.dram_tensor("idb", [128, 128], BF16, kind="ExternalInput")
    # [L, group, 128, hc-major (wq|wk|wv) 384] fp16
    wqkv_h = nc.dram_tensor("wqkv", [L, 2, 128, KH * 384], F16, kind="ExternalInput")
    # [L, 128, fc-major 2048] fp16
    wo_h = nc.dram_tensor("wo", [L, 128, 2 * 2048], F16, kind="ExternalInput")
    # [L, m-chunk, 128, hc-major (w1|w3) 256] fp16
    w13_h = nc.dram_tensor("w13", [L, 6, 128, KH * 256], F16, kind="ExternalInput")
    # [L, hb, 128, kc-major 512] fp16
    w2_h = nc.dram_tensor("w2", [L, 4, 128, 6 * 512], F16, kind="ExternalInput")
    # [hc, 128, VC] fp16 (final norm weight folded in)
    ow_h = nc.dram_tensor("ow", [KH, 128, VC], F16, kind="ExternalInput")
    out_h = nc.dram_tensor("logits", [1, VC], F32, kind="ExternalOutput")

    from contextlib import ExitStack

    with tile.TileContext(nc) as tc, ExitStack() as _ctx:
        ec = _ctx.enter_context
        p_const = ec(tc.tile_pool(name="consts", bufs=1))
        p_resid = ec(tc.tile_pool(name="resid", bufs=1))
        p_q = ec(tc.tile_pool(name="qp", bufs=1))
        p_k = ec(tc.tile_pool(name="kp", bufs=1))
        p_v = ec(tc.tile_pool(name="vp", bufs=1))
        p_attn = ec(tc.tile_pool(name="attnp", bufs=1))
        p_swig = ec(tc.tile_pool(name="swigp", bufs=2))
        p_bc = ec(tc.tile_pool(name="bcp", bufs=3))
        p_sq = ec(tc.tile_pool(name="sqp", bufs=2))
        p_pt = ec(tc.tile_pool(name="ptp", bufs=3))
        p_ex = ec(tc.tile_pool(name="exp", bufs=2))
        p_ib = ec(tc.tile_pool(name="ibp", bufs=1))
        p_fs = ec(tc.tile_pool(name="fsp", bufs=2))
        p_rt = ec(tc.tile_pool(name="rtp", bufs=1))
        p_vt = ec(tc.tile_pool(name="vtp", bufs=2))
        p_row = ec(tc.tile_pool(name="rowp", bufs=2))
        p_mini = ec(tc.tile_pool(name="minip", bufs=3))
        p_a2 = ec(tc.tile_pool(name="a2p", bufs=2))
        p_art = ec(tc.tile_pool(name="artp", bufs=2))
        p_wqkv = ec(tc.tile_pool(name="wqkvp", bufs=3))
        p_wo = ec(tc.tile_pool(name="wop", bufs=2))
        p_w13 = ec(tc.tile_pool(name="w13p", bufs=3))
        p_w2 = ec(tc.tile_pool(name="w2p", bufs=2))
        p_ow = ec(tc.tile_pool(name="owp", bufs=6))
        p_l2 = ec(tc.tile_pool(name="l2p", bufs=2))
        psum = ec(tc.tile_pool(name="psum", bufs=8, space="PSUM"))
        dram = ec(tc.tile_pool(name="dram", bufs=8, space="DRAM"))

        # ---------------- constants ----------------
        xT = p_resid.tile([128, KH * S], F16, tag="xT")
        for c4 in range(4):
            nc.sync.dma_start(xT[:, c4 * 4 * S: (c4 + 1) * 4 * S],
                              xT_h.ap()[:, c4 * 4 * S: (c4 + 1) * 4 * S])
        mask4 = p_const.tile([128, 4 * 512], BF16, tag="mask4")
        nc.sync.dma_start(mask4[:], mask_h.ap())
        C_s = p_const.tile([128, S], BF16, tag="C")
        nc.sync.dma_start(C_s[:], C_h.ap())
        S_s = p_const.tile([128, S], BF16, tag="S")
        nc.sync.dma_start(S_s[:], S_h.ap())
        J_b = p_const.tile([128, 128], BF16, tag="J")
        nc.sync.dma_start(J_b[:], J_h.ap())
        id_b = p_const.tile([128, 128], BF16, tag="id")
        nc.sync.dma_start(id_b[:], id_h.ap())

        ones_f = p_const.tile([128, 1], F32, tag="o1f")
        nc.vector.memset(ones_f[:], 1.0)
        ones_col_b = p_const.tile([128, 1], BF16, tag="o1b")
        nc.vector.tensor_copy(ones_col_b[:], ones_f[:])
        ones_rf = p_const.tile([1, 128], F32, tag="orf")
        nc.vector.memset(ones_rf[:], 1.0)
        ones_row_r = p_const.tile([1, 128], F32R, tag="orr")
        nc.vector.tensor_copy(ones_row_r[:], ones_rf[:])
        eps_t = p_const.tile([1, 1], F32, tag="eps")
        nc.vector.memset(eps_t[:], EPS)

        # ---------------- helpers ----------------
        def norm_bc(tk):
            """[128,512] f32 tile holding 1/rms for tokens [tk*512, +512)."""
            accs = [p_sq.tile([128, 512], BF16, tag="sacc", name=f"sacc{i}")
                    for i in range(2)]
            for hc in range(KH):
                sl = slice(hc * S + tk * 512, hc * S + tk * 512 + 512)
                acc = accs[hc // 8]
                if hc % 8 == 0:
                    nc.vector.tensor_mul(acc[:], xT[:, sl], xT[:, sl])
                else:
                    sq = p_sq.tile([128, 512], BF16, tag="sq", name="sq")
                    nc.vector.tensor_mul(sq[:], xT[:, sl], xT[:, sl])
                    nc.vector.tensor_add(acc[:], acc[:], sq[:])
            ss = psum.tile([1, 512], F32, tag="ps", name="ss")
            nc.tensor.matmul(ss[:], ones_col_b[:], accs[0][:], start=True, stop=False)
            nc.tensor.matmul(ss[:], ones_col_b[:], accs[1][:], start=False, stop=True)
            row = p_row.tile([1, 512], F32R, tag="row", name="nrow")
            nc.scalar.activation(row[:], ss[:], AF.Sqrt, bias=eps_t[:], scale=1.0 / H)
            bc_ps = psum.tile([128, 512], F32, tag="ps", name="bcp")
            nc.tensor.matmul(bc_ps[:], ones_row_r[:], row[:], start=True, stop=True)
            bc = p_bc.tile([128, 512], F32, tag="bc", name="bc")
            nc.vector.reciprocal_approx_fast(bc[:], bc_ps[:])
            return bc

        def qkv_pass(l, tk, q_s, k_s, v_s, do_q=True, q2s=None):
            """QKV projections for token half tk; writes transposed q/k and
            natural-layout v. If q2s is given (last layer), also computes the
            last-2-token q into q2s [128, 4]."""
            bc = norm_bc(tk)
            for g in range(2):
                wta = p_wqkv.tile([128, 8 * 384], F16, tag="wqkv", name="wta")
                nc.scalar.dma_start(wta[:], wqkv_h.ap()[l, g, :, 0: 8 * 384])
                wtb = p_wqkv.tile([128, 8 * 384], F16, tag="wqkv", name="wtb")
                nc.scalar.dma_start(wtb[:], wqkv_h.ap()[l, g, :, 8 * 384: KH * 384])
                if do_q:
                    qp = psum.tile([128, 512], F32, tag="ps", name="qp")
                kp = psum.tile([128, 512], F32, tag="ps", name="kp")
                vp = psum.tile([128, 512], F32, tag="ps", name="vp")
                if q2s is not None:
                    q2p = psum.tile([128, 2], F32, tag="ps", name="q2p")
                for hc in range(KH):
                    mv = xT[:, hc * S + tk * 512: hc * S + tk * 512 + 512]
                    st, sp = (hc == 0), (hc == KH - 1)
                    wt = wta if hc < 8 else wtb
                    o = (hc % 8) * 384
                    if do_q:
                        nc.tensor.matmul(qp[:], wt[:, o: o + 128], mv,
                                         start=st, stop=sp)
                    nc.tensor.matmul(kp[:], wt[:, o + 128: o + 256], mv,
                                     start=st, stop=sp)
                    nc.tensor.matmul(vp[:], wt[:, o + 256: o + 384], mv,
                                     start=st, stop=sp)
                    if q2s is not None:
                        nc.tensor.matmul(q2p[:], wt[:, o: o + 128],
                                         xT[:, hc * S + S - 2: hc * S + S],
                                         start=st, stop=sp)
                dst = slice(g * S + tk * 512, g * S + tk * 512 + 512)
                if do_q:
                    nc.vector.tensor_mul(q_s[:, dst], qp[:], bc[:])
                nc.vector.tensor_mul(k_s[:, dst], kp[:], bc[:])
                vt = p_vt.tile([128, 512], BF16, tag="vt", name="vt")
                nc.vector.tensor_mul(vt[:], vp[:], bc[:])
                for tb in range(4):
                    tp = psum.tile([128, 128], BF16, tag="ps", name="tp")
                    nc.tensor.transpose(tp[:], vt[:, tb * 128: tb * 128 + 128],
                                        id_b[:])
                    o2 = (tk * 4 + tb) * FEAT + g * 128
                    nc.vector.tensor_copy(v_s[:, o2: o2 + 128], tp[:])
                if q2s is not None:
                    # scale by bc for tokens 1022/1023 (tk must be 1)
                    nc.vector.tensor_mul(q2s[:, g * 2: g * 2 + 2], q2p[:],
                                         bc[:, 510:512])

        def rope(t_s, g, tk):
            sl = slice(g * S + tk * 512, g * S + tk * 512 + 512)
            csl = slice(tk * 512, tk * 512 + 512)
            j_ps = psum.tile([128, 512], F32, tag="ps", name="jps")
            nc.tensor.matmul(j_ps[:], J_b[:], t_s[:, sl], start=True, stop=True)
            tmp = p_rt.tile([128, 512], BF16, tag="rt", name="rt")
            nc.vector.tensor_mul(tmp[:], C_s[:, csl], t_s[:, sl])
            nc.vector.tensor_mul(t_s[:, sl], j_ps[:], S_s[:, csl])
            nc.vector.tensor_add(t_s[:, sl], t_s[:, sl], tmp[:])

        def attn_block(tk, h, q_s, k_s, v_s, attn_s):
            ncc = 4 * (tk + 1)
            at = psum.tile([128, 512], F32, tag="ps", name="at")
            rs = psum.tile([1, 512], F32, tag="ps", name="rs")
            for kc in range(ncc):
                sc = psum.tile([128, 512], F32, tag="ps", name="sc")
                nc.tensor.matmul(
                    sc[:], k_s[:, h * S + kc * 128: h * S + kc * 128 + 128],
                    q_s[:, h * S + tk * 512: h * S + tk * 512 + 512],
                    start=True, stop=True)
                pt = p_pt.tile([128, 512], BF16, tag="pt", name="pt")
                if kc >= 4 * tk:
                    j = kc - 4 * tk
                    ex = p_ex.tile([128, 512], F32, tag="ex", name="ex")
                    nc.vector.scalar_tensor_tensor(
                        ex[:], sc[:], INV_SCALE,
                        mask4[:, j * 512: j * 512 + 512],
                        op0=ALU.mult, op1=ALU.add)
                    nc.scalar.activation(pt[:], ex[:], AF.Exp)
                else:
                    nc.scalar.activation(pt[:], sc[:], AF.Exp, scale=INV_SCALE)
                st, sp = (kc == 0), (kc == ncc - 1)
                nc.tensor.matmul(
                    at[:], v_s[:, kc * FEAT + h * 128: kc * FEAT + h * 128 + 128],
                    pt[:], start=st, stop=sp)
                nc.tensor.matmul(rs[:], ones_col_b[:], pt[:], start=st, stop=sp)
            row = p_row.tile([1, 512], F32R, tag="row", name="srow")
            nc.vector.tensor_copy(row[:], rs[:])
            ib_ps = psum.tile([128, 512], F32, tag="ps", name="ibp")
            nc.tensor.matmul(ib_ps[:], ones_row_r[:], row[:], start=True, stop=True)
            ib = p_ib.tile([128, 512], F32, tag="ib", name="ib")
            nc.vector.reciprocal_approx_fast(ib[:], ib_ps[:])
            nc.vector.tensor_mul(
                attn_s[:, h * S + tk * 512: h * S + tk * 512 + 512], at[:], ib[:])

        def wo_half(l, tk, wo_t, attn_s, ar_in, half):
            for hp in (2 * half, 2 * half + 1):
                a2 = p_a2.tile([128, 4 * 512], F16, tag="a2", name="a2")
                for hh in range(4):
                    po = psum.tile([128, 512], F32, tag="ps", name="po")
                    col = (hp * 4 + hh) * 128
                    for fc in range(2):
                        nc.tensor.matmul(
                            po[:], wo_t[:, fc * 2048 + col: fc * 2048 + col + 128],
                            attn_s[:, fc * S + tk * 512: fc * S + tk * 512 + 512],
                            start=(fc == 0), stop=(fc == 1))
                    nc.vector.tensor_copy(a2[:, hh * 512: hh * 512 + 512], po[:])
                row0 = (hp - 2 * half) * 512
                nc.sync.dma_start(
                    ar_in[row0: row0 + 512, :]
                    .rearrange("(c p) f -> p c f", p=128),
                    a2[:].rearrange("p (c f) -> p c f", c=4))

        def ar_add(tk, ar_pair):
            """Read back a pair of half-AllReduce results and add into xT."""
            for hb in range(4):
                ar_out = ar_pair[hb // 2]
                art = p_art.tile([128, 4 * 512], F16, tag="art", name="art")
                row0 = (hb % 2) * 512
                nc.sync.dma_start(
                    art[:].rearrange("p (c f) -> p c f", c=4),
                    ar_out[row0: row0 + 512, :]
                    .rearrange("(c p) f -> p c f", p=128))
                for j in range(4):
                    hc = hb * 4 + j
                    sl = slice(hc * S + tk * 512, hc * S + tk * 512 + 512)
                    nc.vector.tensor_add(xT[:, sl], xT[:, sl],
                                         art[:, j * 512: j * 512 + 512])

        def ffn(l, tk):
            bc2 = norm_bc(tk)
            swig = p_swig.tile([128, 6 * 512], BF16, tag="swig", name="swig")
            for m in range(6):
                w13a = p_w13.tile([128, 8 * 256], F16, tag="w13", name="w13a")
                nc.scalar.dma_start(w13a[:], w13_h.ap()[l, m, :, 0: 8 * 256])
                w13b = p_w13.tile([128, 8 * 256], F16, tag="w13", name="w13b")
                nc.scalar.dma_start(w13b[:], w13_h.ap()[l, m, :, 8 * 256: KH * 256])
                gp = psum.tile([128, 512], F32, tag="ps", name="gp")
                up = psum.tile([128, 512], F32, tag="ps", name="up")
                for hc in range(KH):
                    mv = xT[:, hc * S + tk * 512: hc * S + tk * 512 + 512]
                    st, sp = (hc == 0), (hc == KH - 1)
                    w13t = w13a if hc < 8 else w13b
                    o = (hc % 8) * 256
                    nc.tensor.matmul(gp[:], w13t[:, o: o + 128],
                                     mv, start=st, stop=sp)
                    nc.tensor.matmul(up[:], w13t[:, o + 128: o + 256],
                                     mv, start=st, stop=sp)
                t1 = p_fs.tile([128, 512], BF16, tag="fs", name="t1")
                nc.vector.tensor_mul(t1[:], gp[:], bc2[:])
                gs = p_fs.tile([128, 512], BF16, tag="fs", name="gs")
                nc.scalar.activation(gs[:], t1[:], AF.Silu)
                t2 = p_fs.tile([128, 512], BF16, tag="fs", name="t2")
                nc.vector.tensor_mul(t2[:], gs[:], up[:])
                nc.vector.tensor_mul(swig[:, m * 512: m * 512 + 512], t2[:], bc2[:])
            def w2_half(ar_in, half):
                for hb in (2 * half, 2 * half + 1):
                    w2t = p_w2.tile([128, 6 * 512], F16, tag="w2", name="w2t")
                    nc.scalar.dma_start(w2t[:], w2_h.ap()[l, hb])
                    a2 = p_a2.tile([128, 4 * 512], F16, tag="a2", name="a2f")
                    for hh in range(4):
                        p2 = psum.tile([128, 512], F32, tag="ps", name="p2")
                        for kc in range(6):
                            nc.tensor.matmul(
                                p2[:],
                                w2t[:, kc * 512 + hh * 128: kc * 512 + hh * 128 + 128],
                                swig[:, kc * 512: kc * 512 + 512],
                                start=(kc == 0), stop=(kc == 5))
                        nc.vector.tensor_copy(a2[:, hh * 512: hh * 512 + 512],
                                              p2[:])
                    row0 = (hb % 2) * 512
                    nc.sync.dma_start(
                        ar_in[row0: row0 + 512, :]
                        .rearrange("(c p) f -> p c f", p=128),
                        a2[:].rearrange("p (c f) -> p c f", c=4))
            return w2_half

        def all_reduce(ar_in, ar_out):
            nc.gpsimd.collective_compute(
                "AllReduce", ALU.add, replica_groups=[list(range(NC))],
                ins=[ar_in[:].opt()], outs=[ar_out[:].opt()])

        # ---------------- main pipeline ----------------
        RG = list(range(NC))
        qkv_tiles = {}
        ar1 = {}
        ar2 = {}

        def new_qkv(l):
            q_s = p_q.tile([128, 2 * S], BF16, tag="q", name=f"q{l}")
            k_s = p_k.tile([128, 2 * S], BF16, tag="k", name=f"k{l}")
            v_s = p_v.tile([128, 2 * S], BF16, tag="v", name=f"v{l}")
            attn_s = p_attn.tile([128, 2 * S], BF16, tag="attn", name=f"at{l}")
            return q_s, k_s, v_s, attn_s

        def A_chain(l, tk, wo_t):
            """QKV -> rope -> attn -> wo -> AR1 issue, for (l, tk)."""
            q_s, k_s, v_s, attn_s = qkv_tiles[l]
            if l > 0:
                ar_add(tk, ar2[(l - 1, tk)])
            qkv_pass(l, tk, q_s, k_s, v_s)
            for g in range(2):
                rope(q_s, g, tk)
                rope(k_s, g, tk)
            for h in range(2):
                attn_block(tk, h, q_s, k_s, v_s, attn_s)
            outs = []
            for half in range(2):
                ar_in = dram.tile([H // 2, 512], F16, tag="arin",
                                  name=f"ai{l}{tk}{half}")
                ar_out = dram.tile([H // 2, 512], F16, tag="arout",
                                   addr_space="Shared", name=f"ao{l}{tk}{half}")
                wo_half(l, tk, wo_t, attn_s, ar_in, half)
                all_reduce(ar_in, ar_out)
                outs.append(ar_out)
            ar1[(l, tk)] = outs

        def F_chain(l, tk):
            """AR1 add -> norm2 -> SwiGLU -> w2 -> AR2 issue, for (l, tk)."""
            ar_add(tk, ar1[(l, tk)])
            w2_half = ffn(l, tk)
            outs = []
            for half in range(2):
                ar_in = dram.tile([H // 2, 512], F16, tag="arin",
                                  name=f"a2i{l}{tk}{half}")
                ar_out = dram.tile([H // 2, 512], F16, tag="arout",
                                   addr_space="Shared", name=f"a2o{l}{tk}{half}")
                w2_half(ar_in, half)
                all_reduce(ar_in, ar_out)
                outs.append(ar_out)
            ar2[(l, tk)] = outs

        for l in range(L - 1):
            qkv_tiles[l] = new_qkv(l)
            wo_t = p_wo.tile([128, 2 * 2048], F16, tag="wo", name=f"wo{l}")
            nc.scalar.dma_start(wo_t[:], wo_h.ap()[l])
            A_chain(l, 0, wo_t)
            A_chain(l, 1, wo_t)
            F_chain(l, 0)
            F_chain(l, 1)

        # ---------------- last layer ----------------
        l = L - 1
        k_s = p_k.tile([128, 2 * S], BF16, tag="k", name=f"k{l}")
        v_s = p_v.tile([128, 2 * S], BF16, tag="v", name=f"v{l}")
        q_s = None
        q2s = p_mini.tile([128, 4], BF16, tag="q2s", name="q2s")

        ar_add(0, ar2[(l - 1, 0)])
        qkv_pass(l, 0, q_s, k_s, v_s, do_q=False)
        for g in range(2):
            rope(k_s, g, 0)
        ar_add(1, ar2[(l - 1, 1)])
        qkv_pass(l, 1, q_s, k_s, v_s, do_q=False, q2s=q2s)
        for g in range(2):
            rope(k_s, g, 1)
        # prefetch output-weight chunks during the last layer; fetch order is
        # vocab-half-major to match the two logits accumulation passes
        ow_tiles = {}
        OW_ORDER = [hc * 2 + half for half in range(2) for hc in range(KH)]

        def ow_fetch(pos):
            idx = OW_ORDER[pos]
            hc, half = idx // 2, idx % 2
            t = p_ow.tile([128, 2000], F16, tag="ow", name=f"ow{idx}")
            nc.scalar.dma_start(t[:], ow_h.ap()[hc, :, half * 2000: half * 2000 + 2000])
            ow_tiles[idx] = t

        for pos in range(6):
            ow_fetch(pos)

        # rope on the 2-token q
        for g in range(2):
            j2 = psum.tile([128, 2], F32, tag="ps", name="j2")
            nc.tensor.matmul(j2[:], J_b[:], q2s[:, g * 2: g * 2 + 2],
                             start=True, stop=True)
            tm2 = p_mini.tile([128, 2], BF16, tag="tm2", name="tm2")
            nc.vector.tensor_mul(tm2[:], C_s[:, S - 2: S], q2s[:, g * 2: g * 2 + 2])
            nc.vector.tensor_mul(q2s[:, g * 2: g * 2 + 2], j2[:], S_s[:, S - 2: S])
            nc.vector.tensor_add(q2s[:, g * 2: g * 2 + 2],
                                 q2s[:, g * 2: g * 2 + 2], tm2[:])

        # attention for the last 2 tokens: one [2,1024] transposed-score
        # matmul per head, softmax sums via activation accum_out, probs
        # normalized in transposed space then transposed back for AV.
        mask2T = p_const.tile([2, S], BF16, tag="mask2T", name="mask2T")
        nc.vector.memset(mask2T[:], 0.0)
        nc.vector.memset(mask2T[0:1, S - 1: S], -30000.0)
        attn2 = p_mini.tile([128, 4], BF16, tag="attn2", name="attn2")
        for h in range(2):
            pt2T = p_l2.tile([2, S], BF16, tag="pt2T", name="pt2T")
            rs2a = p_mini.tile([2, 2], F32, tag="rs2a", name="rs2a")
            for piece in range(2):
                scT = psum.tile([2, 512], F32, tag="ps", name="scT")
                nc.tensor.matmul(
                    scT[:], q2s[:, h * 2: h * 2 + 2],
                    k_s[:, h * S + piece * 512: h * S + piece * 512 + 512],
                    start=True, stop=True)
                ex2T = p_l2.tile([2, 512], BF16, tag="ex2T", name="ex2T")
                nc.vector.scalar_tensor_tensor(
                    ex2T[:], scT[:], INV_SCALE,
                    mask2T[:, piece * 512: piece * 512 + 512],
                    op0=ALU.mult, op1=ALU.add)
                nc.scalar.activation(pt2T[:, piece * 512: piece * 512 + 512],
                                     ex2T[:], AF.Exp,
                                     accum_out=rs2a[:, piece: piece + 1])
            rsum = p_mini.tile([2, 1], F32, tag="rs2b", name="rsum")
            nc.vector.reduce_sum(rsum[:], rs2a[:], axis=mybir.AxisListType.X)
            inv2 = p_mini.tile([2, 1], F32, tag="rs2b", name="inv2")
            nc.vector.reciprocal_approx_fast(inv2[:], rsum[:])
            pn2T = p_l2.tile([2, S], BF16, tag="pt2T", name="pn2T")
            nc.vector.tensor_scalar_mul(pn2T[:], pt2T[:], inv2[:])
            pn2 = p_mini.tile([128, 16], BF16, tag="pn2", name="pn2")
            for kc in range(8):
                tp2 = psum.tile([128, 2], BF16, tag="ps", name="tp2")
                nc.tensor.transpose(tp2[:], pn2T[:, kc * 128: kc * 128 + 128],
                                    id_b[0:2, 0:2])
                nc.vector.tensor_copy(pn2[:, kc * 2: kc * 2 + 2], tp2[:])
            at2 = psum.tile([128, 2], F32, tag="ps", name="at2")
            for kc in range(8):
                nc.tensor.matmul(
                    at2[:], v_s[:, kc * FEAT + h * 128: kc * FEAT + h * 128 + 128],
                    pn2[:, kc * 2: kc * 2 + 2], start=(kc == 0), stop=(kc == 7))
            nc.vector.tensor_copy(attn2[:, h * 2: h * 2 + 2], at2[:])

        # wo for last 2 tokens -> AllReduce -> residual add
        wo_t = p_wo.tile([128, 2 * 2048], F16, tag="wo", name="woL")
        nc.scalar.dma_start(wo_t[:], wo_h.ap()[l])
        a2L = p_mini.tile([128, KH * 2], F16, tag="a2L", name="a2L")
        for hc in range(KH):
            poL = psum.tile([128, 2], F32, tag="ps", name="poL")
            for fc in range(2):
                nc.tensor.matmul(
                    poL[:], wo_t[:, fc * 2048 + hc * 128: fc * 2048 + hc * 128 + 128],
                    attn2[:, fc * 2: fc * 2 + 2], start=(fc == 0), stop=(fc == 1))
            nc.vector.tensor_copy(a2L[:, hc * 2: hc * 2 + 2], poL[:])
        arL_in = dram.tile([H, 2], F16, tag="arinL", name="arLi")
        arL_out = dram.tile([H, 2], F16, tag="aroutL", addr_space="Shared",
                            name="arLo")
        nc.sync.dma_start(
            arL_in[:].rearrange("(c p) f -> p c f", p=128),
            a2L[:].rearrange("p (c f) -> p c f", c=KH))
        all_reduce(arL_in, arL_out)
        artL = p_mini.tile([128, KH * 2], F16, tag="a2L", name="artL")
        nc.sync.dma_start(
            artL[:].rearrange("p (c f) -> p c f", c=KH),
            arL_out[:].rearrange("(c p) f -> p c f", p=128))
        for hc in range(KH):
            sl = slice(hc * S + S - 2, hc * S + S)
            nc.vector.tensor_add(xT[:, sl], xT[:, sl],
                                 artL[:, hc * 2: hc * 2 + 2])

        # norm2 for last 2 tokens: bc2L [128,2] = 1/rms broadcast
        sq2 = p_mini.tile([128, KH * 2], BF16, tag="sq2", name="sq2")
        for hc in range(KH):
            sl = slice(hc * S + S - 2, hc * S + S)
            nc.vector.tensor_mul(sq2[:, hc * 2: hc * 2 + 2], xT[:, sl], xT[:, sl])
        ss2 = psum.tile([1, KH * 2], F32, tag="ps", name="ss2")
        nc.tensor.matmul(ss2[:], ones_col_b[:], sq2[:], start=True, stop=True)
        rowa = p_row.tile([1, KH * 2], F32R, tag="row", name="rowa")
        nc.vector.tensor_copy(rowa[:], ss2[:])
        rowb = p_mini.tile([1, 2], F32R, tag="row2", name="rowb")
        with nc.allow_low_precision(reason="f32r has fp32 bits"):
            nc.vector.reduce_sum(
                rowb[:], rowa[:].rearrange("p (c two) -> p two c", two=2),
                axis=mybir.AxisListType.X)
        rms2 = p_mini.tile([1, 2], F32R, tag="row2", name="rms2")
        nc.scalar.activation(rms2[:], rowb[:], AF.Sqrt, bias=eps_t[:], scale=1.0 / H)
        bc2_ps = psum.tile([128, 2], F32, tag="ps", name="bc2p")
        nc.tensor.matmul(bc2_ps[:], ones_row_r[:], rms2[:], start=True, stop=True)
        bc2L = p_mini.tile([128, 2], F32, tag="ib2", name="bc2L")
        nc.vector.reciprocal_approx_fast(bc2L[:], bc2_ps[:])

        # SwiGLU for last 2 tokens
        swig2 = p_mini.tile([128, 12], BF16, tag="swig2", name="swig2")
        for m in range(6):
            w13a = p_w13.tile([128, 8 * 256], F16, tag="w13", name="w13La")
            nc.scalar.dma_start(w13a[:], w13_h.ap()[l, m, :, 0: 8 * 256])
            w13b = p_w13.tile([128, 8 * 256], F16, tag="w13", name="w13Lb")
            nc.scalar.dma_start(w13b[:], w13_h.ap()[l, m, :, 8 * 256: KH * 256])
            g2 = psum.tile([128, 2], F32, tag="ps", name="g2")
            u2 = psum.tile([128, 2], F32, tag="ps", name="u2")
            for hc in range(KH):
                mv = xT[:, hc * S + S - 2: hc * S + S]
                st, sp = (hc == 0), (hc == KH - 1)
                w13t = w13a if hc < 8 else w13b
                o = (hc % 8) * 256
                nc.tensor.matmul(g2[:], w13t[:, o: o + 128], mv,
                                 start=st, stop=sp)
                nc.tensor.matmul(u2[:], w13t[:, o + 128: o + 256],
                                 mv, start=st, stop=sp)
            t1L = p_mini.tile([128, 2], BF16, tag="t1L", name="t1L")
            nc.vector.tensor_mul(t1L[:], g2[:], bc2L[:])
            gsL = p_mini.tile([128, 2], BF16, tag="gsL", name="gsL")
            nc.scalar.activation(gsL[:], t1L[:], AF.Silu)
            t2L = p_mini.tile([128, 2], BF16, tag="t2L", name="t2L")
            nc.vector.tensor_mul(t2L[:], gsL[:], u2[:])
            nc.vector.tensor_mul(swig2[:, m * 2: m * 2 + 2], t2L[:], bc2L[:])
        a2L2 = p_mini.tile([128, KH * 2], F16, tag="a2L", name="a2L2")
        for hb in range(4):
            w2t = p_w2.tile([128, 6 * 512], F16, tag="w2", name="w2L")
            nc.scalar.dma_start(w2t[:], w2_h.ap()[l, hb])
            for hh in range(4):
                p2 = psum.tile([128, 2], F32, tag="ps", name="p2L")
                for kc in range(6):
                    nc.tensor.matmul(
                        p2[:],
                        w2t[:, kc * 512 + hh * 128: kc * 512 + hh * 128 + 128],
                        swig2[:, kc * 2: kc * 2 + 2],
                        start=(kc == 0), stop=(kc == 5))
                hc = hb * 4 + hh
                nc.vector.tensor_copy(a2L2[:, hc * 2: hc * 2 + 2], p2[:])
        ar2L_in = dram.tile([H, 2], F16, tag="arinL", name="ar2Li")
        ar2L_out = dram.tile([H, 2], F16, tag="aroutL", addr_space="Shared",
                             name="ar2Lo")
        nc.sync.dma_start(
            ar2L_in[:].rearrange("(c p) f -> p c f", p=128),
            a2L2[:].rearrange("p (c f) -> p c f", c=KH))
        all_reduce(ar2L_in, ar2L_out)
        artL2 = p_mini.tile([128, KH * 2], F16, tag="a2L", name="artL2")
        nc.sync.dma_start(
            artL2[:].rearrange("p (c f) -> p c f", c=KH),
            ar2L_out[:].rearrange("(c p) f -> p c f", p=128))
        for hc in range(KH):
            sl = slice(hc * S + S - 2, hc * S + S)
            nc.vector.tensor_add(xT[:, sl], xT[:, sl],
                                 artL2[:, hc * 2: hc * 2 + 2])

        # ---------------- final norm + logits ----------------
        sql = p_mini.tile([128, KH], BF16, tag="sql", name="sql")
        for hc in range(KH):
            sl = slice(hc * S + S - 1, hc * S + S)
            nc.vector.tensor_mul(sql[:, hc: hc + 1], xT[:, sl], xT[:, sl])
        ssl = psum.tile([1, KH], F32, tag="ps", name="ssl")
        nc.tensor.matmul(ssl[:], ones_col_b[:], sql[:], start=True, stop=True)
        rowl = p_row.tile([1, KH], F32R, tag="row", name="rowl")
        nc.vector.tensor_copy(rowl[:], ssl[:])
        ssc = p_mini.tile([1, 1], F32, tag="ssc", name="ssc")
        nc.vector.reduce_sum(ssc[:], rowl[:], axis=mybir.AxisListType.X)
        rmsl = p_mini.tile([1, 1], F32, tag="ssc", name="rmsl")
        nc.scalar.activation(rmsl[:], ssc[:], AF.Sqrt, bias=eps_t[:], scale=1.0 / H)
        invl = p_mini.tile([1, 1], F32, tag="ssc", name="invl")
        nc.vector.reciprocal_approx_fast(invl[:], rmsl[:])

        for half in range(2):
            lgs = [psum.tile([1, 500], F32, tag="ps", name=f"lg{half}{nn}")
                   for nn in range(4)]
            for hc in range(KH):
                owt = ow_tiles[hc * 2 + half]
                for nn in range(4):
                    nc.tensor.matmul(lgs[nn][:], xT[:, hc * S + S - 1: hc * S + S],
                                     owt[:, nn * 500: nn * 500 + 500],
                                     start=(hc == 0), stop=(hc == KH - 1))
                pos = half * KH + hc
                if pos + 6 < 2 * KH:
                    ow_fetch(pos + 6)
            for nn in range(4):
                n = half * 4 + nn
                lg = p_row.tile([1, 500], F32, tag="row", name="lgout")
                nc.scalar.activation(lg[:], lgs[nn][:], AF.Copy, scale=invl[:])
                nc.sync.dma_start(out_h.ap()[:, n * 500: n * 500 + 500], lg[:])

    nc.compile()
    return nc


def _shard(inputs):
    f16 = np.float16
    x = np.asarray(inputs["x"], np.float32)
    mask = np.asarray(inputs["attn_mask"], np.float32)
    cos = np.asarray(inputs["cos"], np.float32).reshape(S, HD // 2)
    sin = np.asarray(inputs["sin"], np.float32).reshape(S, HD // 2)
    n1 = np.asarray(inputs["norm1_w"], np.float32)[:L]
    n2 = np.asarray(inputs["norm2_w"], np.float32)[:L]
    fw = np.asarray(inputs["final_norm_w"], np.float32)
    wq = np.asarray(inputs["wq"], np.float32)[:L] * n1[:, None, :]
    wk = np.asarray(inputs["wk"], np.float32)[:L] * n1[:, None, :]
    wv = np.asarray(inputs["wv"], np.float32)[:L] * n1[:, None, :]
    wo = np.asarray(inputs["wo"], np.float32)[:L]
    w1 = np.asarray(inputs["w1"], np.float32)[:L] * n2[:, None, :]
    w3 = np.asarray(inputs["w3"], np.float32)[:L] * n2[:, None, :]
    w2 = np.asarray(inputs["w2"], np.float32)[:L]
    ow = np.asarray(inputs["out_w"], np.float32) * fw[None, :]

    # residual, transposed + chunked: [128, hc*S + tok]
    xT = np.ascontiguousarray(
        x[0].T.reshape(KH, 128, S).transpose(1, 0, 2)
        .reshape(128, KH * S)).astype(f16)
    # diagonal mask blocks [128, j*512 + q] (shared by both token halves)
    maskT = mask[0].T  # [k, q]
    m4 = np.empty((128, 4 * 512), np.float32)
    for j in range(4):
        m4[:, j * 512:(j + 1) * 512] = maskT[j * 128:(j + 1) * 128, 0:512]
    m4 = np.maximum(m4, -30000.0)  # keep exp()==0 while fitting in fp16
    C = np.empty((128, S), np.float32)
    C[0::2] = cos.T
    C[1::2] = cos.T
    Sm = np.empty((128, S), np.float32)
    Sm[0::2] = -sin.T
    Sm[1::2] = sin.T
    J = np.zeros((128, 128), np.float32)
    idx = np.arange(0, 128, 2)
    J[idx, idx + 1] = 1.0
    J[idx + 1, idx] = 1.0
    ident = np.eye(128, dtype=np.float32)

    common = dict(xT=xT, mask4=m4.astype(f16), Cb=C.astype(f16),
                  Sb=Sm.astype(f16), Jb=J.astype(f16), idb=ident.astype(f16))
    in_maps = []
    for c in range(NC):
        fs = slice(c * FEAT, (c + 1) * FEAT)
        ps = slice(c * PC, (c + 1) * PC)
        vs = slice(c * VC, (c + 1) * VC)
        m = dict(common)

        # wqkv: [L, 2, 128, hc*384 + (q|k|v)*128 + col]
        wqT = wq[:, fs, :].transpose(0, 2, 1)  # [L, H, FEAT]
        wkT = wk[:, fs, :].transpose(0, 2, 1)
        wvT = wv[:, fs, :].transpose(0, 2, 1)
        wqkv = np.empty((L, 2, 128, KH * 384), np.float32)
        for g in range(2):
            gq = wqT[:, :, g * 128:(g + 1) * 128]  # [L, H, 128]
            gk = wkT[:, :, g * 128:(g + 1) * 128]
            gv = wvT[:, :, g * 128:(g + 1) * 128]
            blk = np.concatenate([
                gq.reshape(L, KH, 128, 128),
                gk.reshape(L, KH, 128, 128),
                gv.reshape(L, KH, 128, 128)], axis=3)  # [L, KH, 128, 384]
            wqkv[:, g] = blk.transpose(0, 2, 1, 3).reshape(L, 128, KH * 384)
        m["wqkv"] = wqkv.astype(f16)

        # wo: [L, 128, fc*2048 + col]
        woT = wo[:, :, fs].transpose(0, 2, 1)  # [L, FEAT, H]
        m["wo"] = np.ascontiguousarray(
            woT.reshape(L, 2, 128, H).transpose(0, 2, 1, 3)
            .reshape(L, 128, 2 * H)).astype(f16)

        # w13: [L, m, 128, hc*256 + (w1|w3)*128]  (PC padded to 768)
        w1p = np.zeros((L, PCP, H), np.float32)
        w1p[:, :PC] = w1[:, ps, :]
        w3p = np.zeros((L, PCP, H), np.float32)
        w3p[:, :PC] = w3[:, ps, :]
        w1T = w1p.transpose(0, 2, 1).reshape(L, KH, 128, 6, 128)
        w3T = w3p.transpose(0, 2, 1).reshape(L, KH, 128, 6, 128)
        w13 = np.concatenate([w1T, w3T], axis=4)  # [L, KH, 128, 6, 256]
        m["w13"] = np.ascontiguousarray(
            w13.transpose(0, 3, 2, 1, 4).reshape(L, 6, 128, KH * 256)).astype(f16)

        # w2: [L, hb, 128, kc*512 + col]
        w2p = np.zeros((L, H, PCP), np.float32)
        w2p[:, :, :PC] = w2[:, :, ps]
        w2T = w2p.transpose(0, 2, 1)  # [L, PCP, H]
        w2r = w2T.reshape(L, 6, 128, 4, 512)
        m["w2"] = np.ascontiguousarray(
            w2r.transpose(0, 3, 2, 1, 4).reshape(L, 4, 128, 6 * 512)).astype(f16)

        # ow: [hc, 128, VC]
        owT = ow[vs, :].T  # [H, VC]
        m["ow"] = np.ascontiguousarray(owT.reshape(KH, 128, VC)).astype(f16)
        in_maps.append(m)
    return in_maps


def kernel(**inputs) -> np.ndarray:
    from concourse import bass_utils

    if "nc" not in _STATE:
        _STATE["nc"] = _build()
    in_maps = _shard(inputs)
    res = bass_utils.run_bass_kernel_spmd(
        _STATE["nc"], in_maps, core_ids=list(range(NC)))
    out = np.concatenate(
        [res.results[c]["logits"] for c in range(NC)], axis=1)
    return out.astype(np.float32)


# revision 23
# speedup vs baseline: 1.0076x; 1.0076x over previous
"""Trainium2 Bass kernel: 4-layer decoder prefill (S=1024, H=2048, NH=16, HD=128,
FFN=5632, V=32000), tensor-parallel over 8 NeuronCores.

v2 design:
- Megatron TP over 8 cores (2 heads / 704 ffn rows / 4000 vocab rows per core).
- Weights converted to fp16 on host (FWL-eligible, half the HBM traffic) and
  pre-chunked so every weight DMA is one large contiguous [128, N] transfer,
  loaded once per use-pass.
- RMSNorm weights folded into wq/wk/wv/w1/w3/out_w on host; the per-token
  1/rms scale is broadcast via a ones-matmul and fused into the PSUM->SBUF
  consume ops, so normalization adds no dedicated passes.
- Causal block-skip: score blocks entirely above the diagonal are skipped,
  blocks fully below skip the mask add; only the 4 diagonal 128x512 mask
  blocks (shared by both token halves) are kept in SBUF.
- Software pipeline over (layer, token-half) chains: A = QKV->RoPE->attn->
  wo->AllReduce1, F = resid-add->norm->SwiGLU->w2->AllReduce2, emitted so
  ~35us of independent PE work covers every collective.
- All on-chip activations and weights are fp16 (PSUM accumulation in
  fp32); per-token 1/rms and softmax 1/sum broadcasts stay fp32.
"""

import os
import sys

sys.path.insert(0, "/opt/trn_rl_repo")

import numpy as np

L, B, S, H, NH, HD = 4, 1, 1024, 2048, 16, 128
V, P = 32000, 5632
NC = 8
FEAT = H // NC          # 256 qkv features per core (2 heads of 128)
PC = P // NC            # 704 ffn rows per core
PCP = 768               # padded to 6 chunks of 128
VC = V // NC            # 4000 vocab rows per core
KH = H // 128           # 16 H-chunks
EPS = 1e-5
INV_SCALE = float(1.0 / np.sqrt(HD))

_STATE = {}


def _build():
    import concourse.bass as bass
    import concourse.bacc as bacc
    from concourse import tile, mybir

    F32 = mybir.dt.float32
    F32R = mybir.dt.float32r
    F16 = mybir.dt.float16
    BF16 = mybir.dt.float16  # all 16-bit on-chip data is fp16
    AF = mybir.ActivationFunctionType
    ALU = mybir.AluOpType

    nc = bacc.Bacc("TRN2", target_bir_lowering=False, debug=False, num_devices=NC)

    xT_h = nc.dram_tensor("xT", [128, KH * S], F16, kind="ExternalInput")
    mask_h = nc.dram_tensor("mask4", [128, 4 * 512], BF16, kind="ExternalInput")
    C_h = nc.dram_tensor("Cb", [128, S], BF16, kind="ExternalInput")
    S_h = nc.dram_tensor("Sb", [128, S], BF16, kind="ExternalInput")
    J_h = nc.dram_tensor("Jb", [128, 128], BF16, kind="ExternalInput")
    id_h = nc.dram_tensor("idb", [128, 128], BF16, kind="ExternalInput")
    # [L, group, 128, hc-major (wq|wk|wv) 384] fp16
    wqkv_h = nc.dram_tensor("wqkv", [L, 2, 128, KH * 384], F16, kind="ExternalInput")
    # [L, 128, fc-major 2048] fp16
    wo_h = nc.dram_tensor("wo", [L, 128, 2 * 2048], F16, kind="ExternalInput")
    # [L, m-chunk, 128, hc-major (w1|w3) 256] fp16
    w13_h = nc.dram_tensor("w13", [L, 6, 128, KH * 256], F16, kind="ExternalInput")
    # [L, hb, 128, kc-major 512] fp16
    w2_h = nc.dram_tensor("w2", [L, 4, 128, 6 * 512], F16, kind="ExternalInput")
    # [hc, 128, VC] fp16 (final norm weight folded in)
    ow_h = nc.dram_tensor("ow", [KH, 128, VC], F16, kind="ExternalInput")
    out_h = nc.dram_tensor("logits", [1, VC], F32, kind="ExternalOutput")

    from contextlib import ExitStack

    with tile.TileContext(nc) as tc, ExitStack() as _ctx:
        ec = _ctx.enter_context
        p_const = ec(tc.tile_pool(name="consts", bufs=1))
        p_resid = ec(tc.tile_pool(name="resid", bufs=1))
        p_q = ec(tc.tile_pool(name="qp", bufs=1))
        p_k = ec(tc.tile_pool(name="kp", bufs=1))
        p_v = ec(tc.tile_pool(name="vp", bufs=1))
        p_attn = ec(tc.tile_pool(name="attnp", bufs=1))
        p_swig = ec(tc.tile_pool(name="swigp", bufs=2))
        p_bc = ec(tc.tile_pool(name="bcp", bufs=3))
        p_sq = ec(tc.tile_pool(name="sqp", bufs=2))
        p_pt = ec(tc.tile_pool(name="ptp", bufs=3))
        p_ex = ec(tc.tile_pool(name="exp", bufs=2))
        p_ib = ec(tc.tile_pool(name="ibp", bufs=1))
        p_fs = ec(tc.tile_pool(name="fsp", bufs=2))
        p_rt = ec(tc.tile_pool(name="rtp", bufs=1))
        p_vt = ec(tc.tile_pool(name="vtp", bufs=2))
        p_row = ec(tc.tile_pool(name="rowp", bufs=2))
        p_mini = ec(tc.tile_pool(name="minip", bufs=3))
        p_a2 = ec(tc.tile_pool(name="a2p", bufs=2))
        p_art = ec(tc.tile_pool(name="artp", bufs=2))
        p_wqkv = ec(tc.tile_pool(name="wqkvp", bufs=3))
        p_wo = ec(tc.tile_pool(name="wop", bufs=2))
        p_w13 = ec(tc.tile_pool(name="w13p", bufs=3))
        p_w2 = ec(tc.tile_pool(name="w2p", bufs=2))
        p_ow = ec(tc.tile_pool(name="owp", bufs=6))
        p_l2 = ec(tc.tile_pool(name="l2p", bufs=2))
        psum = ec(tc.tile_pool(name="psum", bufs=8, space="PSUM"))
        dram = ec(tc.tile_pool(name="dram", bufs=8, space="DRAM"))

        # ---------------- constants ----------------
        xT = p_resid.tile([128, KH * S], F16, tag="xT")
        for c4 in range(4):
            nc.sync.dma_start(xT[:, c4 * 4 * S: (c4 + 1) * 4 * S],
                              xT_h.ap()[:, c4 * 4 * S: (c4 + 1) * 4 * S])
        mask4 = p_const.tile([128, 4 * 512], BF16, tag="mask4")
        nc.sync.dma_start(mask4[:], mask_h.ap())
        C_s = p_const.tile([128, S], BF16, tag="C")
        nc.sync.dma_start(C_s[:], C_h.ap())
        S_s = p_const.tile([128, S], BF16, tag="S")
        nc.sync.dma_start(S_s[:], S_h.ap())
        J_b = p_const.tile([128, 128], BF16, tag="J")
        nc.sync.dma_start(J_b[:], J_h.ap())
        id_b = p_const.tile([128, 128], BF16, tag="id")
        nc.sync.dma_start(id_b[:], id_h.ap())

        ones_f = p_const.tile([128, 1], F32, tag="o1f")
        nc.vector.memset(ones_f[:], 1.0)
        ones_col_b = p_const.tile([128, 1], BF16, tag="o1b")
        nc.vector.tensor_copy(ones_col_b[:], ones_f[:])
        ones_rf = p_const.tile([1, 128], F32, tag="orf")
        nc.vector.memset(ones_rf[:], 1.0)
        ones_row_r = p_const.tile([1, 128], F32R, tag="orr")
        nc.vector.tensor_copy(ones_row_r[:], ones_rf[:])
        eps_t = p_const.tile([1, 1], F32, tag="eps")
        nc.vector.memset(eps_t[:], EPS)

        # ---------------- helpers ----------------
        def norm_bc(tk):
            """[128,512] f32 tile holding 1/rms for tokens [tk*512, +512)."""
            acc = p_sq.tile([128, 512], BF16, tag="sacc", name="sacc")
            for hc in range(KH):
                sl = slice(hc * S + tk * 512, hc * S + tk * 512 + 512)
                if hc == 0:
                    nc.vector.tensor_mul(acc[:], xT[:, sl], xT[:, sl])
                else:
                    sq = p_sq.tile([128, 512], BF16, tag="sq", name="sq")
                    nc.vector.tensor_mul(sq[:], xT[:, sl], xT[:, sl])
                    nc.vector.tensor_add(acc[:], acc[:], sq[:])
            ss = psum.tile([1, 512], F32, tag="ps", name="ss")
            nc.tensor.matmul(ss[:], ones_col_b[:], acc[:], start=True, stop=True)
            row = p_row.tile([1, 512], F32R, tag="row", name="nrow")
            nc.scalar.activation(row[:], ss[:], AF.Sqrt, bias=eps_t[:], scale=1.0 / H)
            bc_ps = psum.tile([128, 512], F32, tag="ps", name="bcp")
            nc.tensor.matmul(bc_ps[:], ones_row_r[:], row[:], start=True, stop=True)
            bc = p_bc.tile([128, 512], F32, tag="bc", name="bc")
            nc.vector.reciprocal_approx_fast(bc[:], bc_ps[:])
            return bc

        def qkv_pass(l, tk, q_s, k_s, v_s, do_q=True, q2s=None):
            """QKV projections for token half tk; writes transposed q/k and
            natural-layout v. If q2s is given (last layer), also computes the
            last-2-token q into q2s [128, 4]."""
            bc = norm_bc(tk)
            for g in range(2):
                wta = p_wqkv.tile([128, 8 * 384], F16, tag="wqkv", name="wta")
                nc.scalar.dma_start(wta[:], wqkv_h.ap()[l, g, :, 0: 8 * 384])
                wtb = p_wqkv.tile([128, 8 * 384], F16, tag="wqkv", name="wtb")
                nc.scalar.dma_start(wtb[:], wqkv_h.ap()[l, g, :, 8 * 384: KH * 384])
                if do_q:
                    qp = psum.tile([128, 512], F32, tag="ps", name="qp")
                kp = psum.tile([128, 512], F32, tag="ps", name="kp")
                vp = psum.tile([128, 512], F32, tag="ps", name="vp")
                if q2s is not None:
                    q2p = psum.tile([128, 2], F32, tag="ps", name="q2p")
                for hc in range(KH):
                    mv = xT[:, hc * S + tk * 512: hc * S + tk * 512 + 512]
                    st, sp = (hc == 0), (hc == KH - 1)
                    wt = wta if hc < 8 else wtb
                    o = (hc % 8) * 384
                    if do_q:
                        nc.tensor.matmul(qp[:], wt[:, o: o + 128], mv,
                                         start=st, stop=sp)
                    nc.tensor.matmul(kp[:], wt[:, o + 128: o + 256], mv,
                                     start=st, stop=sp)
                    nc.tensor.matmul(vp[:], wt[:, o + 256: o + 384], mv,
                                     start=st, stop=sp)
                    if q2s is not None:
                        nc.tensor.matmul(q2p[:], wt[:, o: o + 128],
                                         xT[:, hc * S + S - 2: hc * S + S],
                                         start=st, stop=sp)
                dst = slice(g * S + tk * 512, g * S + tk * 512 + 512)
                if do_q:
                    nc.vector.tensor_mul(q_s[:, dst], qp[:], bc[:])
                nc.vector.tensor_mul(k_s[:, dst], kp[:], bc[:])
                vt = p_vt.tile([128, 512], BF16, tag="vt", name="vt")
                nc.vector.tensor_mul(vt[:], vp[:], bc[:])
                for tb in range(4):
                    tp = psum.tile([128, 128], BF16, tag="ps", name="tp")
                    nc.tensor.transpose(tp[:], vt[:, tb * 128: tb * 128 + 128],
                                        id_b[:])
                    o2 = (tk * 4 + tb) * FEAT + g * 128
                    nc.vector.tensor_copy(v_s[:, o2: o2 + 128], tp[:])
                if q2s is not None:
                    # scale by bc for tokens 1022/1023 (tk must be 1)
                    nc.vector.tensor_mul(q2s[:, g * 2: g * 2 + 2], q2p[:],
                                         bc[:, 510:512])

        def rope(t_s, g, tk):
            sl = slice(g * S + tk * 512, g * S + tk * 512 + 512)
            csl = slice(tk * 512, tk * 512 + 512)
            j_ps = psum.tile([128, 512], F32, tag="ps", name="jps")
            nc.tensor.matmul(j_ps[:], J_b[:], t_s[:, sl], start=True, stop=True)
            tmp = p_rt.tile([128, 512], BF16, tag="rt", name="rt")
            nc.vector.tensor_mul(tmp[:], C_s[:, csl], t_s[:, sl])
            nc.vector.tensor_mul(t_s[:, sl], j_ps[:], S_s[:, csl])
            nc.vector.tensor_add(t_s[:, sl], t_s[:, sl], tmp[:])

        def attn_block(tk, h, q_s, k_s, v_s, attn_s):
            ncc = 4 * (tk + 1)
            at = psum.tile([128, 512], F32, tag="ps", name="at")
            rs = psum.tile([1, 512], F32, tag="ps", name="rs")
            for kc in range(ncc):
                sc = psum.tile([128, 512], F32, tag="ps", name="sc")
                nc.tensor.matmul(
                    sc[:], k_s[:, h * S + kc * 128: h * S + kc * 128 + 128],
                    q_s[:, h * S + tk * 512: h * S + tk * 512 + 512],
                    start=True, stop=True)
                pt = p_pt.tile([128, 512], BF16, tag="pt", name="pt")
                if kc >= 4 * tk:
                    j = kc - 4 * tk
                    ex = p_ex.tile([128, 512], F32, tag="ex", name="ex")
                    nc.vector.scalar_tensor_tensor(
                        ex[:], sc[:], INV_SCALE,
                        mask4[:, j * 512: j * 512 + 512],
                        op0=ALU.mult, op1=ALU.add)
                    nc.scalar.activation(pt[:], ex[:], AF.Exp)
                else:
                    nc.scalar.activation(pt[:], sc[:], AF.Exp, scale=INV_SCALE)
                st, sp = (kc == 0), (kc == ncc - 1)
                nc.tensor.matmul(
                    at[:], v_s[:, kc * FEAT + h * 128: kc * FEAT + h * 128 + 128],
                    pt[:], start=st, stop=sp)
                nc.tensor.matmul(rs[:], ones_col_b[:], pt[:], start=st, stop=sp)
            row = p_row.tile([1, 512], F32R, tag="row", name="srow")
            nc.vector.tensor_copy(row[:], rs[:])
            ib_ps = psum.tile([128, 512], F32, tag="ps", name="ibp")
            nc.tensor.matmul(ib_ps[:], ones_row_r[:], row[:], start=True, stop=True)
            ib = p_ib.tile([128, 512], F32, tag="ib", name="ib")
            nc.vector.reciprocal_approx_fast(ib[:], ib_ps[:])
            nc.vector.tensor_mul(
                attn_s[:, h * S + tk * 512: h * S + tk * 512 + 512], at[:], ib[:])

        def wo_half(l, tk, wo_t, attn_s, ar_in, half):
            for hp in (2 * half, 2 * half + 1):
                a2 = p_a2.tile([128, 4 * 512], F16, tag="a2", name="a2")
                for hh in range(4):
                    po = psum.tile([128, 512], F32, tag="ps", name="po")
                    col = (hp * 4 + hh) * 128
                    for fc in range(2):
                        nc.tensor.matmul(
                            po[:], wo_t[:, fc * 2048 + col: fc * 2048 + col + 128],
                            attn_s[:, fc * S + tk * 512: fc * S + tk * 512 + 512],
                            start=(fc == 0), stop=(fc == 1))
                    nc.vector.tensor_copy(a2[:, hh * 512: hh * 512 + 512], po[:])
                row0 = (hp - 2 * half) * 512
                nc.sync.dma_start(
                    ar_in[row0: row0 + 512, :]
                    .rearrange("(c p) f -> p c f", p=128),
                    a2[:].rearrange("p (c f) -> p c f", c=4))

        def ar_add(tk, ar_pair):
            """Read back a pair of half-AllReduce results and add into xT."""
            for hb in range(4):
                ar_out = ar_pair[hb // 2]
                art = p_art.tile([128, 4 * 512], F16, tag="art", name="art")
                row0 = (hb % 2) * 512
                nc.sync.dma_start(
                    art[:].rearrange("p (c f) -> p c f", c=4),
                    ar_out[row0: row0 + 512, :]
                    .rearrange("(c p) f -> p c f", p=128))
                for j in range(4):
                    hc = hb * 4 + j
                    sl = slice(hc * S + tk * 512, hc * S + tk * 512 + 512)
                    nc.vector.tensor_add(xT[:, sl], xT[:, sl],
                                         art[:, j * 512: j * 512 + 512])

        def ffn(l, tk):
            bc2 = norm_bc(tk)
            swig = p_swig.tile([128, 6 * 512], BF16, tag="swig", name="swig")
            for m in range(6):
                w13a = p_w13.tile([128, 8 * 256], F16, tag="w13", name="w13a")
                nc.scalar.dma_start(w13a[:], w13_h.ap()[l, m, :, 0: 8 * 256])
                w13b = p_w13.tile([128, 8 * 256], F16, tag="w13", name="w13b")
                nc.scalar.dma_start(w13b[:], w13_h.ap()[l, m, :, 8 * 256: KH * 256])
                gp = psum.tile([128, 512], F32, tag="ps", name="gp")
                up = psum.tile([128, 512], F32, tag="ps", name="up")
                for hc in range(KH):
                    mv = xT[:, hc * S + tk * 512: hc * S + tk * 512 + 512]
                    st, sp = (hc == 0), (hc == KH - 1)
                    w13t = w13a if hc < 8 else w13b
                    o = (hc % 8) * 256
                    nc.tensor.matmul(gp[:], w13t[:, o: o + 128],
                                     mv, start=st, stop=sp)
                    nc.tensor.matmul(up[:], w13t[:, o + 128: o + 256],
                                     mv, start=st, stop=sp)
                t1 = p_fs.tile([128, 512], BF16, tag="fs", name="t1")
                nc.vector.tensor_mul(t1[:], gp[:], bc2[:])
                gs = p_fs.tile([128, 512], BF16, tag="fs", name="gs")
                nc.scalar.activation(gs[:], t1[:], AF.Silu)
                t2 = p_fs.tile([128, 512], BF16, tag="fs", name="t2")
                nc.vector.tensor_mul(t2[:], gs[:], up[:])
                nc.vector.tensor_mul(swig[:, m * 512: m * 512 + 512], t2[:], bc2[:])
            def w2_half(ar_in, half):
                for hb in (2 * half, 2 * half + 1):
                    w2t = p_w2.tile([128, 6 * 512], F16, tag="w2", name="w2t")
                    nc.scalar.dma_start(w2t[:], w2_h.ap()[l, hb])
                    a2 = p_a2.tile([128, 4 * 512], F16, tag="a2", name="a2f")
                    for hh in range(4):
                        p2 = psum.tile([128, 512], F32, tag="ps", name="p2")
                        for kc in range(6):
                            nc.tensor.matmul(
                                p2[:],
                                w2t[:, kc * 512 + hh * 128: kc * 512 + hh * 128 + 128],
                                swig[:, kc * 512: kc * 512 + 512],
                                start=(kc == 0), stop=(kc == 5))
                        nc.vector.tensor_copy(a2[:, hh * 512: hh * 512 + 512],
                                              p2[:])
                    row0 = (hb % 2) * 512
                    nc.sync.dma_start(
                        ar_in[row0: row0 + 512, :]
                        .rearrange("(c p) f -> p c f", p=128),
                        a2[:].rearrange("p (c f) -> p c f", c=4))
            return w2_half

        def all_reduce(ar_in, ar_out):
            nc.gpsimd.collective_compute(
                "AllReduce", ALU.add, replica_groups=[list(range(NC))],
                ins=[ar_in[:].opt()], outs=[ar_out[:].opt()])

        # ---------------- main pipeline ----------------
        RG = list(range(NC))
        qkv_tiles = {}
        ar1 = {}
        ar2 = {}

        def new_qkv(l):
            q_s = p_q.tile([128, 2 * S], BF16, tag="q", name=f"q{l}")
            k_s = p_k.tile([128, 2 * S], BF16, tag="k", name=f"k{l}")
            v_s = p_v.tile([128, 2 * S], BF16, tag="v", name=f"v{l}")
            attn_s = p_attn.tile([128, 2 * S], BF16, tag="attn", name=f"at{l}")
            return q_s, k_s, v_s, attn_s

        def A_chain(l, tk, wo_t):
            """QKV -> rope -> attn -> wo -> AR1 issue, for (l, tk)."""
            q_s, k_s, v_s, attn_s = qkv_tiles[l]
            if l > 0:
                ar_add(tk, ar2[(l - 1, tk)])
            qkv_pass(l, tk, q_s, k_s, v_s)
            for g in range(2):
                rope(q_s, g, tk)
                rope(k_s, g, tk)
            for h in range(2):
                attn_block(tk, h, q_s, k_s, v_s, attn_s)
            outs = []
            for half in range(2):
                ar_in = dram.tile([H // 2, 512], F16, tag="arin",
                                  name=f"ai{l}{tk}{half}")
                ar_out = dram.tile([H // 2, 512], F16, tag="arout",
                                   addr_space="Shared", name=f"ao{l}{tk}{half}")
                wo_half(l, tk, wo_t, attn_s, ar_in, half)
                all_reduce(ar_in, ar_out)
                outs.append(ar_out)
            ar1[(l, tk)] = outs

        def F_chain(l, tk):
            """AR1 add -> norm2 -> SwiGLU -> w2 -> AR2 issue, for (l, tk)."""
            ar_add(tk, ar1[(l, tk)])
            w2_half = ffn(l, tk)
            outs = []
            for half in range(2):
                ar_in = dram.tile([H // 2, 512], F16, tag="arin",
                                  name=f"a2i{l}{tk}{half}")
                ar_out = dram.tile([H // 2, 512], F16, tag="arout",
                                   addr_space="Shared", name=f"a2o{l}{tk}{half}")
                w2_half(ar_in, half)
                all_reduce(ar_in, ar_out)
                outs.append(ar_out)
            ar2[(l, tk)] = outs

        for l in range(L - 1):
            qkv_tiles[l] = new_qkv(l)
            wo_t = p_wo.tile([128, 2 * 2048], F16, tag="wo", name=f"wo{l}")
            nc.scalar.dma_start(wo_t[:], wo_h.ap()[l])
            A_chain(l, 0, wo_t)
            A_chain(l, 1, wo_t)
            F_chain(l, 0)
            F_chain(l, 1)

        # ---------------- last layer ----------------
        l = L - 1
        k_s = p_k.tile([128, 2 * S], BF16, tag="k", name=f"k{l}")
        v_s = p_v.tile([128, 2 * S], BF16, tag="v", name=f"v{l}")
        q_s = None
        q2s = p_mini.tile([128, 4], BF16, tag="q2s", name="q2s")

        ar_add(0, ar2[(l - 1, 0)])
        qkv_pass(l, 0, q_s, k_s, v_s, do_q=False)
        for g in range(2):
            rope(k_s, g, 0)
        ar_add(1, ar2[(l - 1, 1)])
        qkv_pass(l, 1, q_s, k_s, v_s, do_q=False, q2s=q2s)
        for g in range(2):
            rope(k_s, g, 1)
        # prefetch output-weight chunks during the last layer; fetch order is
        # vocab-half-major to match the two logits accumulation passes
        ow_tiles = {}
        OW_ORDER = [hc * 2 + half for half in range(2) for hc in range(KH)]

        def ow_fetch(pos):
            idx = OW_ORDER[pos]
            hc, half = idx // 2, idx % 2
            t = p_ow.tile([128, 2000], F16, tag="ow", name=f"ow{idx}")
            nc.scalar.dma_start(t[:], ow_h.ap()[hc, :, half * 2000: half * 2000 + 2000])
            ow_tiles[idx] = t

        for pos in range(6):
            ow_fetch(pos)

        # rope on the 2-token q
        for g in range(2):
            j2 = psum.tile([128, 2], F32, tag="ps", name="j2")
            nc.tensor.matmul(j2[:], J_b[:], q2s[:, g * 2: g * 2 + 2],
                             start=True, stop=True)
            tm2 = p_mini.tile([128, 2], BF16, tag="tm2", name="tm2")
            nc.vector.tensor_mul(tm2[:], C_s[:, S - 2: S], q2s[:, g * 2: g * 2 + 2])
            nc.vector.tensor_mul(q2s[:, g * 2: g * 2 + 2], j2[:], S_s[:, S - 2: S])
            nc.vector.tensor_add(q2s[:, g * 2: g * 2 + 2],
                                 q2s[:, g * 2: g * 2 + 2], tm2[:])

        # attention for the last 2 tokens: one [2,1024] transposed-score
        # matmul per head, softmax sums via activation accum_out, probs
        # normalized in transposed space then transposed back for AV.
        mask2T = p_const.tile([2, S], BF16, tag="mask2T", name="mask2T")
        nc.vector.memset(mask2T[:], 0.0)
        nc.vector.memset(mask2T[0:1, S - 1: S], -30000.0)
        attn2 = p_mini.tile([128, 4], BF16, tag="attn2", name="attn2")
        for h in range(2):
            pt2T = p_l2.tile([2, S], BF16, tag="pt2T", name="pt2T")
            rs2a = p_mini.tile([2, 2], F32, tag="rs2a", name="rs2a")
            for piece in range(2):
                scT = psum.tile([2, 512], F32, tag="ps", name="scT")
                nc.tensor.matmul(
                    scT[:], q2s[:, h * 2: h * 2 + 2],
                    k_s[:, h * S + piece * 512: h * S + piece * 512 + 512],
                    start=True, stop=True)
                ex2T = p_l2.tile([2, 512], BF16, tag="ex2T", name="ex2T")
                nc.vector.scalar_tensor_tensor(
                    ex2T[:], scT[:], INV_SCALE,
                    mask2T[:, piece * 512: piece * 512 + 512],
                    op0=ALU.mult, op1=ALU.add)
                nc.scalar.activation(pt2T[:, piece * 512: piece * 512 + 512],
                                     ex2T[:], AF.Exp,
                                     accum_out=rs2a[:, piece: piece + 1])
            rsum = p_mini.tile([2, 1], F32, tag="rs2b", name="rsum")
            nc.vector.reduce_sum(rsum[:], rs2a[:], axis=mybir.AxisListType.X)
            inv2 = p_mini.tile([2, 1], F32, tag="rs2b", name="inv2")
            nc.vector.reciprocal_approx_fast(inv2[:], rsum[:])
            pn2T = p_l2.tile([2, S], BF16, tag="pt2T", name="pn2T")
            nc.vector.tensor_scalar_mul(pn2T[:], pt2T[:], inv2[:])
            pn2 = p_mini.tile([128, 16], BF16, tag="pn2", name="pn2")
            for kc in range(8):
                tp2 = psum.tile([128, 2], BF16, tag="ps", name="tp2")
                nc.tensor.transpose(tp2[:], pn2T[:, kc * 128: kc * 128 + 128],
                                    id_b[0:2, 0:2])
                nc.vector.tensor_copy(pn2[:, kc * 2: kc * 2 + 2], tp2[:])
            at2 = psum.tile([128, 2], F32, tag="ps", name="at2")
            for kc in range(8):
                nc.tensor.matmul(
                    at2[:], v_s[:, kc * FEAT + h * 128: kc * FEAT + h * 128 + 128],
                    pn2[:, kc * 2: kc * 2 + 2], start=(kc == 0), stop=(kc == 7))
            nc.vector.tensor_copy(attn2[:, h * 2: h * 2 + 2], at2[:])

        # wo for last 2 tokens -> AllReduce -> residual add
        wo_t = p_wo.tile([128, 2 * 2048], F16, tag="wo", name="woL")
        nc.scalar.dma_start(wo_t[:], wo_h.ap()[l])
        a2L = p_mini.tile([128, KH * 2], F16, tag="a2L", name="a2L")
        for hc in range(KH):
            poL = psum.tile([128, 2], F32, tag="ps", name="poL")
            for fc in range(2):
                nc.tensor.matmul(
                    poL[:], wo_t[:, fc * 2048 + hc * 128: fc * 2048 + hc * 128 + 128],
                    attn2[:, fc * 2: fc * 2 + 2], start=(fc == 0), stop=(fc == 1))
            nc.vector.tensor_copy(a2L[:, hc * 2: hc * 2 + 2], poL[:])
        arL_in = dram.tile([H, 2], F16, tag="arinL", name="arLi")
        arL_out = dram.tile([H, 2], F16, tag="aroutL", addr_space="Shared",
                            name="arLo")
        nc.sync.dma_start(
            arL_in[:].rearrange("(c p) f -> p c f", p=128),
            a2L[:].rearrange("p (c f) -> p c f", c=KH))
        all_reduce(arL_in, arL_out)
        artL = p_mini.tile([128, KH * 2], F16, tag="a2L", name="artL")
        nc.sync.dma_start(
            artL[:].rearrange("p (c f) -> p c f", c=KH),
            arL_out[:].rearrange("(c p) f -> p c f", p=128))
        for hc in range(KH):
            sl = slice(hc * S + S - 2, hc * S + S)
            nc.vector.tensor_add(xT[:, sl], xT[:, sl],
                                 artL[:, hc * 2: hc * 2 + 2])

        # norm2 for last 2 tokens: bc2L [128,2] = 1/rms broadcast
        sq2 = p_mini.tile([128, KH * 2], BF16, tag="sq2", name="sq2")
        for hc in range(KH):
            sl = slice(hc * S + S - 2, hc * S + S)
            nc.vector.tensor_mul(sq2[:, hc * 2: hc * 2 + 2], xT[:, sl], xT[:, sl])
        ss2 = psum.tile([1, KH * 2], F32, tag="ps", name="ss2")
        nc.tensor.matmul(ss2[:], ones_col_b[:], sq2[:], start=True, stop=True)
        rowa = p_row.tile([1, KH * 2], F32R, tag="row", name="rowa")
        nc.vector.tensor_copy(rowa[:], ss2[:])
        rowb = p_mini.tile([1, 2], F32R, tag="row2", name="rowb")
        with nc.allow_low_precision(reason="f32r has fp32 bits"):
            nc.vector.reduce_sum(
                rowb[:], rowa[:].rearrange("p (c two) -> p two c", two=2),
                axis=mybir.AxisListType.X)
        rms2 = p_mini.tile([1, 2], F32R, tag="row2", name="rms2")
        nc.scalar.activation(rms2[:], rowb[:], AF.Sqrt, bias=eps_t[:], scale=1.0 / H)
        bc2_ps = psum.tile([128, 2], F32, tag="ps", name="bc2p")
        nc.tensor.matmul(bc2_ps[:], ones_row_r[:], rms2[:], start=True, stop=True)
        bc2L = p_mini.tile([128, 2], F32, tag="ib2", name="bc2L")
        nc.vector.reciprocal_approx_fast(bc2L[:], bc2_ps[:])

        # SwiGLU for last 2 tokens
        swig2 = p_mini.tile([128, 12], BF16, tag="swig2", name="swig2")
        for m in range(6):
            w13a = p_w13.tile([128, 8 * 256], F16, tag="w13", name="w13La")
            nc.scalar.dma_start(w13a[:], w13_h.ap()[l, m, :, 0: 8 * 256])
            w13b = p_w13.tile([128, 8 * 256], F16, tag="w13", name="w13Lb")
            nc.scalar.dma_start(w13b[:], w13_h.ap()[l, m, :, 8 * 256: KH * 256])
            g2 = psum.tile([128, 2], F32, tag="ps", name="g2")
            u2 = psum.tile([128, 2], F32, tag="ps", name="u2")
            for hc in range(KH):
                mv = xT[:, hc * S + S - 2: hc * S + S]
                st, sp = (hc == 0), (hc == KH - 1)
                w13t = w13a if hc < 8 else w13b
                o = (hc % 8) * 256
                nc.tensor.matmul(g2[:], w13t[:, o: o + 128], mv,
                                 start=st, stop=sp)
                nc.tensor.matmul(u2[:], w13t[:, o + 128: o + 256],
                                 mv, start=st, stop=sp)
            t1L = p_mini.tile([128, 2], BF16, tag="t1L", name="t1L")
            nc.vector.tensor_mul(t1L[:], g2[:], bc2L[:])
            gsL = p_mini.tile([128, 2], BF16, tag="gsL", name="gsL")
            nc.scalar.activation(gsL[:], t1L[:], AF.Silu)
            t2L = p_mini.tile([128, 2], BF16, tag="t2L", name="t2L")
            nc.vector.tensor_mul(t2L[:], gsL[:], u2[:])
            nc.vector.tensor_mul(swig2[:, m * 2: m * 2 + 2], t2L[:], bc2L[:])
        a2L2 = p_mini.tile([128, KH * 2], F16, tag="a2L", name="a2L2")
        for hb in range(4):
            w2t = p_w2.tile([128, 6 * 512], F16, tag="w2", name="w2L")
            nc.scalar.dma_start(w2t[:], w2_h.ap()[l, hb])
            for hh in range(4):
                p2 = psum.tile([128, 2], F32, tag="ps", name="p2L")
                for kc in range(6):
                    nc.tensor.matmul(
                        p2[:],
                        w2t[:, kc * 512 + hh * 128: kc * 512 + hh * 128 + 128],
                        swig2[:, kc * 2: kc * 2 + 2],
                        start=(kc == 0), stop=(kc == 5))
                hc = hb * 4 + hh
                nc.vector.tensor_copy(a2L2[:, hc * 2: hc * 2 + 2], p2[:])
        ar2L_in = dram.tile([H, 2], F16, tag="arinL", name="ar2Li")
        ar2L_out = dram.tile([H, 2], F16, tag="aroutL", addr_space="Shared",
                             name="ar2Lo")
        nc.sync.dma_start(
            ar2L_in[:].rearrange("(c p) f -> p c f", p=128),
            a2L2[:].rearrange("p (c f) -> p c f", c=KH))
        all_reduce(ar2L_in, ar2L_out)
        artL2 = p_mini.tile([128, KH * 2], F16, tag="a2L", name="artL2")
        nc.sync.dma_start(
            artL2[:].rearrange("p (c f) -> p c f", c=KH),
            ar2L_out[:].rearrange("(c p) f -> p c f", p=128))
        for hc in range(KH):
            sl = slice(hc * S + S - 2, hc * S + S)
            nc.vector.tensor_add(xT[:, sl], xT[:, sl],
                                 artL2[:, hc * 2: hc * 2 + 2])

        # ---------------- final norm + logits ----------------
        sql = p_mini.tile([128, KH], BF16, tag="sql", name="sql")
        for hc in range(KH):
            sl = slice(hc * S + S - 1, hc * S + S)
            nc.vector.tensor_mul(sql[:, hc: hc + 1], xT[:, sl], xT[:, sl])
        ssl = psum.tile([1, KH], F32, tag="ps", name="ssl")
        nc.tensor.matmul(ssl[:], ones_col_b[:], sql[:], start=True, stop=True)
        rowl = p_row.tile([1, KH], F32R, tag="row", name="rowl")
        nc.vector.tensor_copy(rowl[:], ssl[:])
        ssc = p_mini.tile([1, 1], F32, tag="ssc", name="ssc")
        nc.vector.reduce_sum(ssc[:], rowl[:], axis=mybir.AxisListType.X)
        rmsl = p_mini.tile([1, 1], F32, tag="ssc", name="rmsl")
        nc.scalar.activation(rmsl[:], ssc[:], AF.Sqrt, bias=eps_t[:], scale=1.0 / H)
        invl = p_mini.tile([1, 1], F32, tag="ssc", name="invl")
        nc.vector.reciprocal_approx_fast(invl[:], rmsl[:])

        for half in range(2):
            lgs = [psum.tile([1, 500], F32, tag="ps", name=f"lg{half}{nn}")
                   for nn in range(4)]
            for hc in range(KH):
                owt = ow_tiles[hc * 2 + half]
                for nn in range(4):
                    nc.tensor.matmul(lgs[nn][:], xT[:, hc * S + S - 1: hc * S + S],
                                     owt[:, nn * 500: nn * 500 + 500],
                                     start=(hc == 0), stop=(hc == KH - 1))
                pos = half * KH + hc
                if pos + 6 < 2 * KH:
                    ow_fetch(pos + 6)
            for nn in range(4):
                n = half * 4 + nn
                lg = p_row.tile([1, 500], F32, tag="row", name="lgout")
                nc.scalar.activation(lg[:], lgs[nn][:], AF.Copy, scale=invl[:])
                nc.sync.dma_start(out_h.ap()[:, n * 500: n * 500 + 500], lg[:])

    nc.compile()
    return nc


def _shard(inputs):
    f16 = np.float16
    x = np.asarray(inputs["x"], np.float32)
    mask = np.asarray(inputs["attn_mask"], np.float32)
    cos = np.asarray(inputs["cos"], np.float32).reshape(S, HD // 2)
    sin = np.asarray(inputs["sin"], np.float32).reshape(S, HD // 2)
    n1 = np.asarray(inputs["norm1_w"], np.float32)[:L]
    n2 = np.asarray(inputs["norm2_w"], np.float32)[:L]
    fw = np.asarray(inputs["final_norm_w"], np.float32)
    wq = np.asarray(inputs["wq"], np.float32)[:L] * n1[:, None, :]
    wk = np.asarray(inputs["wk"], np.float32)[:L] * n1[:, None, :]
    wv = np.asarray(inputs["wv"], np.float32)[:L] * n1[:, None, :]
    wo = np.asarray(inputs["wo"], np.float32)[:L]
    w1 = np.asarray(inputs["w1"], np.float32)[:L] * n2[:, None, :]
    w3 = np.asarray(inputs["w3"], np.float32)[:L] * n2[:, None, :]
    w2 = np.asarray(inputs["w2"], np.float32)[:L]
    ow = np.asarray(inputs["out_w"], np.float32) * fw[None, :]

    # residual, transposed + chunked: [128, hc*S + tok]
    xT = np.ascontiguousarray(
        x[0].T.reshape(KH, 128, S).transpose(1, 0, 2)
        .reshape(128, KH * S)).astype(f16)
    # diagonal mask blocks [128, j*512 + q] (shared by both token halves)
    maskT = mask[0].T  # [k, q]
    m4 = np.empty((128, 4 * 512), np.float32)
    for j in range(4):
        m4[:, j * 512:(j + 1) * 512] = maskT[j * 128:(j + 1) * 128, 0:512]
    m4 = np.maximum(m4, -30000.0)  # keep exp()==0 while fitting in fp16
    C = np.empty((128, S), np.float32)
    C[0::2] = cos.T
    C[1::2] = cos.T
    Sm = np.empty((128, S), np.float32)
    Sm[0::2] = -sin.T
    Sm[1::2] = sin.T
    J = np.zeros((128, 128), np.float32)
    idx = np.arange(0, 128, 2)
    J[idx, idx + 1] = 1.0
    J[idx + 1, idx] = 1.0
    ident = np.eye(128, dtype=np.float32)

    common = dict(xT=xT, mask4=m4.astype(f16), Cb=C.astype(f16),
                  Sb=Sm.astype(f16), Jb=J.astype(f16), idb=ident.astype(f16))
    in_maps = []
    for c in range(NC):
        fs = slice(c * FEAT, (c + 1) * FEAT)
        ps = slice(c * PC, (c + 1) * PC)
        vs = slice(c * VC, (c + 1) * VC)
        m = dict(common)

        # wqkv: [L, 2, 128, hc*384 + (q|k|v)*128 + col]
        wqT = wq[:, fs, :].transpose(0, 2, 1)  # [L, H, FEAT]
        wkT = wk[:, fs, :].transpose(0, 2, 1)
        wvT = wv[:, fs, :].transpose(0, 2, 1)
        wqkv = np.empty((L, 2, 128, KH * 384), np.float32)
        for g in range(2):
            gq = wqT[:, :, g * 128:(g + 1) * 128]  # [L, H, 128]
            gk = wkT[:, :, g * 128:(g + 1) * 128]
            gv = wvT[:, :, g * 128:(g + 1) * 128]
            blk = np.concatenate([
                gq.reshape(L, KH, 128, 128),
                gk.reshape(L, KH, 128, 128),
                gv.reshape(L, KH, 128, 128)], axis=3)  # [L, KH, 128, 384]
            wqkv[:, g] = blk.transpose(0, 2, 1, 3).reshape(L, 128, KH * 384)
        m["wqkv"] = wqkv.astype(f16)

        # wo: [L, 128, fc*2048 + col]
        woT = wo[:, :, fs].transpose(0, 2, 1)  # [L, FEAT, H]
        m["wo"] = np.ascontiguousarray(
            woT.reshape(L, 2, 128, H).transpose(0, 2, 1, 3)
            .reshape(L, 128, 2 * H)).astype(f16)

        # w13: [L, m, 128, hc*256 + (w1|w3)*128]  (PC padded to 768)
        w1p = np.zeros((L, PCP, H), np.float32)
        w1p[:, :PC] = w1[:, ps, :]
        w3p = np.zeros((L, PCP, H), np.float32)
        w3p[:, :PC] = w3[:, ps, :]
        w1T = w1p.transpose(0, 2, 1).reshape(L, KH, 128, 6, 128)
        w3T = w3p.transpose(0, 2, 1).reshape(L, KH, 128, 6, 128)
        w13 = np.concatenate([w1T, w3T], axis=4)  # [L, KH, 128, 6, 256]
        m["w13"] = np.ascontiguousarray(
            w13.transpose(0, 3, 2, 1, 4).reshape(L, 6, 128, KH * 256)).astype(f16)

        # w2: [L, hb, 128, kc*512 + col]
        w2p = np.zeros((L, H, PCP), np.float32)
        w2p[:, :, :PC] = w2[:, :, ps]
        w2T = w2p.transpose(0, 2, 1)  # [L, PCP, H]
        w2r = w2T.reshape(L, 6, 128, 4, 512)
        m["w2"] = np.ascontiguousarray(
            w2r.transpose(0, 3, 2, 1, 4).reshape(L, 4, 128, 6 * 512)).astype(f16)

        # ow: [hc, 128, VC]
        owT = ow[vs, :].T  # [H, VC]
        m["ow"] = np.ascontiguousarray(owT.reshape(KH, 128, VC)).astype(f16)
        in_maps.append(m)
    return in_maps


def kernel(**inputs) -> np.ndarray:
    from concourse import bass_utils

    if "nc" not in _STATE:
        _STATE["nc"] = _build()
    in_maps = _shard(inputs)
    res = bass_utils.run_bass_kernel_spmd(
        _STATE["nc"], in_maps, core_ids=list(range(NC)))
    out = np.concatenate(
        [res.results[c]["logits"] for c in range(NC)], axis=1)
    return out.astype(np.float32)
